# revision 9
# baseline (speedup 1.0000x reference)
"""MCR2 loss kernel for 8 Trainium2 NeuronCores.

Data-parallel over the sample axis.  The host permutes each core's
75000-row shard of Z so rows are grouped by class (each class padded with
zero rows to a fixed capacity CAP, a multiple of 1024), quantizes to fp8
e4m3, and lays the rows out in the exact SBUF tile layout the device
wants:

    Zb[p, t*32 + f] = rows[t*128 + p, f]

On device, each group of 4 consecutive 128-row tiles forms X = [128, 128]
(4 tiles side by side); one full-width fp8 matmul X^T @ X accumulated in
PSUM yields all four tile-Grams on its 32x32 diagonal blocks (fp8 streams
at bf16 PE speed and the 128-col weights get automatic fast-weight-load).
16 such matmuls per class give
the per-class Gram partial with zero per-sample Vector/Scalar work.  Input
DMAs are posted from SP, output DMAs from the Activation engine so
compute-dependent output posts never stall input streaming.  The host
sums diagonal blocks across cores and evaluates the logdets in float64.
"""

import sys

sys.path.insert(0, "/opt/trn_rl_repo")

import ml_dtypes
import numpy as np

import concourse.bacc as bacc
import concourse.mybir as mybir
import concourse.tile as tile
from concourse.bass_utils import run_bass_kernel_spmd

N, D, C = 600000, 32, 10
EPS = 0.5
NCORES = 8
PER = N // NCORES            # 75000 rows per core
GROUP_ROWS = 512             # 4 tiles of 128 rows -> one [128,128] matmul

_cache = {}


def _build_program(caps):
    """caps: per-class row capacities (each a multiple of 512)."""
    gpcs = [c // GROUP_ROWS for c in caps]   # matmul groups per class
    tiles = sum(caps) // 128                 # 128-row tiles total
    ncols = tiles * 32                       # fp8 bytes per partition of Zb

    fp8 = mybir.dt.float8e4
    f32 = mybir.dt.float32

    nc = bacc.Bacc(None)
    z_dram = nc.dram_tensor("Zb", [128, ncols], fp8, kind="ExternalInput")
    out_dram = nc.dram_tensor("grams", [128, C * 128], f32, kind="ExternalOutput")

    with tile.TileContext(nc) as tc:
        with (
            tc.tile_pool(name="zin", bufs=C) as zin_pool,
            tc.tile_pool(name="outp", bufs=4) as out_pool,
            tc.tile_pool(name="psum", bufs=4, space="PSUM") as psum_pool,
            tc.tile_pool(name="warm", bufs=1) as warm_pool,
            tc.tile_pool(name="wpsum", bufs=1, space="PSUM") as wps_pool,
        ):
            # warm-up matmuls on a zeroed tile while the first chunk is in
            # flight: the PE p-state ramps to full clock after ~3us busy,
            # so burn the slow ramp on dummies instead of real groups
            wsb = warm_pool.tile([128, 128], fp8)
            nc.gpsimd.memset(wsb[:], 0)
            wacc = wps_pool.tile([128, 128], f32)
            NWARM = 28
            for i in range(NWARM):
                nc.tensor.matmul(
                    wacc[:], wsb[:], wsb[:], start=(i == 0), stop=(i == NWARM - 1)
                )
            col = 0
            for j in range(C):
                gpc = gpcs[j]
                acc = psum_pool.tile([128, 128], f32, tag="acc")
                z_sb = zin_pool.tile([128, gpc * 128], fp8, tag="z")
                # input DMAs on the SP queue only: they must never sit
                # behind a post that waits on compute
                nc.sync.dma_start(z_sb[:, : gpc * 128], z_dram[:, col : col + gpc * 128])
                col += gpc * 128
                for g in range(gpc):
                    w = z_sb[:, g * 128 : (g + 1) * 128]
                    nc.tensor.matmul(
                        acc[:],
                        w,
                        w,
                        start=(g == 0),
                        stop=(g == gpc - 1),
                    )
                out_sb = out_pool.tile([128, 128], f32, tag="o")
                nc.vector.tensor_copy(out_sb[:], acc[:])
                # output DMAs posted from the Activation engine (also HWDGE)
                # so their copy-dependent waits don't stall SP input posting
                nc.scalar.dma_start(out_dram[:, j * 128 : (j + 1) * 128], out_sb[:])

    nc.compile()
    return nc


def kernel(Z: np.ndarray, labels: np.ndarray) -> np.ndarray:
    Z = np.asarray(Z, dtype=np.float32)
    labels = np.asarray(labels, dtype=np.int32)

    # per-core class counts decide the (compile-time) class capacity
    counts = np.stack(
        [
            np.bincount(labels[k * PER : (k + 1) * PER], minlength=C)
            for k in range(NCORES)
        ]
    )
    caps = [
        int(np.ceil(counts[:, j].max() / GROUP_ROWS) * GROUP_ROWS) for j in range(C)
    ]
    key = ("nc", tuple(caps))
    if key not in _cache:
        _cache[key] = _build_program(caps)
    nc = _cache[key]

    tiles = sum(caps) // 128
    in_maps = []
    for k in range(NCORES):
        zs = Z[k * PER : (k + 1) * PER]
        ls = labels[k * PER : (k + 1) * PER]
        order = np.argsort(ls, kind="stable")
        srt = zs[order]
        buf = np.zeros([sum(caps), D], dtype=ml_dtypes.float8_e4m3)
        off = 0
        base = 0
        for j in range(C):
            cnt = int(counts[k, j])
            buf[base : base + cnt] = srt[off : off + cnt]
            off += cnt
            base += caps[j]
        zb = np.ascontiguousarray(
            buf.reshape(tiles, 128, D).transpose(1, 0, 2).reshape(128, tiles * D)
        )
        in_maps.append({"Zb": zb})

    res = run_bass_kernel_spmd(nc, in_maps, core_ids=list(range(NCORES)))
    _cache["last_results"] = res

    # host: sum the 4 diagonal 32x32 blocks per class across cores
    gj = np.zeros([C, D, D], np.float64)
    for r in res.results:
        g = r["grams"].astype(np.float64)  # [128, C*128]
        for j in range(C):
            blk = g[:, j * 128 : (j + 1) * 128]
            for a in range(4):
                gj[j] += blk[a * D : (a + 1) * D, a * D : (a + 1) * D]

    g_all = gj.sum(axis=0)
    tr_pi = np.bincount(labels, minlength=C).astype(np.float64)

    nf, df = float(N), float(D)
    eye = np.eye(D)
    loss_r = 0.5 * np.linalg.slogdet(eye + (df / (nf * EPS)) * g_all)[1]
    loss_rc = 0.0
    for j in range(C):
        ld = np.linalg.slogdet(eye + (df / (tr_pi[j] * EPS)) * gj[j])[1]
        loss_rc += (tr_pi[j] / (2.0 * nf)) * ld
    loss_obj = loss_r - loss_rc
    return np.asarray([-loss_obj, loss_r, loss_rc], dtype=np.float32)
